# revision 13
# baseline (speedup 1.0000x reference)
"""Trainium2 Bass kernel for nn_DiWeightedGCNLayer (8-core SPMD).

Math (per reference):
    h   = LayerNorm(x) * gamma + beta
    m   = h @ W.T + b
    msg = m[src] * w
    out = segment_sum(msg, dst) / max(segment_sum(w, dst), 1) * dst_scale
    y   = x + gelu(out)

Sharding: edges sorted by dst, split across 8 cores at node-range
boundaries (core r owns nodes [r*6250, (r+1)*6250)); no collectives.
Each core redundantly computes m for all nodes (LN folded into W on host:
W2[d,d'] = gamma[d]*W[d',d], c = beta@W.T + b), stores m in HBM as bf16.

Phase 2 (v2): per 128-dst-node chunk, TWO bulk dma_gather ops fetch all
the chunk's message rows m[src] at once (m split into two 25024-row
halves because gather indices are int16), then per 128-edge block a fused
DVE op builds a weighted dst-one-hot and a PE matmul scatter-adds into
PSUM (+ weighted degree). This replaces per-block indirect_dma_start
(994ns fixed SWDGE overhead each, serialized on the Pool engine).

Hardware notes (learned the hard way):
- indirect_dma_start dest AP must be 2D; HW consumes ONE offset per
  partition. dma_gather instead takes a wrapped int16 idx list
  ([128, n/16], idx i at partition i%16 (x8 replicas), free i//16) and
  writes row i to out[i%128, i//128, :].
- Build on bacc.Bacc and nc.finalize() before running (wait splitting).
- GPSIMD cannot touch PSUM.
"""

import contextlib
import numpy as np
import ml_dtypes

import concourse.bass as bass
import concourse.bacc as bacc
import concourse.tile as tile
import concourse.mybir as mybir
from concourse.bass_utils import run_bass_kernel_spmd

F32 = mybir.dt.float32
BF16 = mybir.dt.bfloat16
I32 = mybir.dt.int32
I16 = mybir.dt.int16
AF = mybir.ActivationFunctionType
OP = mybir.AluOpType

D = 128
P = 128
LN_EPS = 1e-5


def build_program(geom, loop_n=1, g_tiles=4, n_swdge=4,
                  xp_bufs=4, msg_bufs=8, oh_bufs=6, pst_bufs=1, psm_bufs=1,
                  pso_bufs=2, psd_bufs=2, mp_bufs=4, ht_copy_eng="vector",
                  skip_p1=False, skip_p2=False, skip_gather=False,
                  skip_blocks=False):
    """One-core SPMD program. geom carries: n_pad (padded node rows,
    multiple of 256), nch (node chunks per core), include_c,
    b_lo/b_hi (per-chunk gather block counts, tuples of len nch).
    loop_n>1 repeats the whole computation in-program (benchmarking)."""
    n_pad = geom["n_pad"]
    nch = geom["nch"]
    include_c = geom["include_c"]
    b_lo = geom["b_lo"]
    b_hi = geom["b_hi"]
    half = n_pad // 2
    npc_pad = nch * P
    nt = n_pad // P

    bc = [lo + hi for lo, hi in zip(b_lo, b_hi)]
    bmax_lo = max(b_lo)
    bmax_hi = max(b_hi)
    boff = np.concatenate([[0], np.cumsum(bc)])  # block col offsets
    bsum = int(boff[-1])
    iw_per = [(lo + hi) * 8 for lo, hi in zip(b_lo, b_hi)]
    iwoff = np.concatenate([[0], np.cumsum(iw_per)])
    iwsum = int(iwoff[-1])

    nc = bacc.Bacc(num_swdge_queues=n_swdge)

    x_ext = nc.declare_dram_parameter("x", [n_pad, D], F32, isOutput=False)
    xres_ext = nc.declare_dram_parameter("xres", [npc_pad, D], F32, isOutput=False)
    w2_ext = nc.declare_dram_parameter("w2", [D, D], BF16, isOutput=False)
    iota_ext = nc.declare_dram_parameter("iota", [P, P], BF16, isOutput=False)
    ident_ext = nc.declare_dram_parameter("ident", [P, P], BF16, isOutput=False)
    idx_ext = nc.declare_dram_parameter("idx16", [P, iwsum], I16, isOutput=False)
    relw_ext = nc.declare_dram_parameter("relw", [P, 2 * bsum], F32,
                                         isOutput=False)
    dsc_ext = nc.declare_dram_parameter("dsct", [P, nch], F32, isOutput=False)
    if include_c:
        cb_ext = nc.declare_dram_parameter("cb", [P, D], F32, isOutput=False)
    y_ext = nc.declare_dram_parameter("y", [npc_pad, D], F32, isOutput=True)

    m_dram = nc.dram_tensor("m_scratch", [n_pad, D], BF16)

    with tile.TileContext(nc) as tc:
        with (
            tc.tile_pool(name="const", bufs=1) as const,
            tc.tile_pool(name="xp", bufs=xp_bufs) as xp,
            tc.tile_pool(name="stats", bufs=4) as sp,
            tc.tile_pool(name="small", bufs=6) as smp,
            tc.tile_pool(name="hp", bufs=3) as hp,
            tc.tile_pool(name="htp", bufs=3) as htp,
            tc.tile_pool(name="mp", bufs=mp_bufs) as mp,
            tc.tile_pool(name="msg", bufs=msg_bufs) as msgp,
            tc.tile_pool(name="oh", bufs=oh_bufs) as ohp,
            tc.tile_pool(name="ep", bufs=3) as epp,
            tc.tile_pool(name="ps_t", bufs=pst_bufs, space="PSUM") as ps_t,
            tc.tile_pool(name="ps_m", bufs=psm_bufs, space="PSUM") as ps_m,
            tc.tile_pool(name="ps_o", bufs=pso_bufs, space="PSUM") as ps_o,
            tc.tile_pool(name="ps_d", bufs=psd_bufs, space="PSUM") as ps_d,
        ):
            # --- constants (outside the benchmark loop) ---
            w2_t = const.tile([D, D], BF16)
            nc.sync.dma_start(out=w2_t[:], in_=w2_ext[:, :])
            iota_t = const.tile([P, P], BF16)
            nc.sync.dma_start(out=iota_t[:], in_=iota_ext[:, :])
            ident = const.tile([P, P], BF16)
            nc.sync.dma_start(out=ident[:], in_=ident_ext[:, :])
            ones_t = const.tile([P, 1], BF16)
            nc.vector.memset(ones_t[:], 1.0)
            eps_t = const.tile([P, 1], F32)
            nc.vector.memset(eps_t[:], LN_EPS)
            dsc_t = const.tile([P, nch], F32)
            nc.sync.dma_start(out=dsc_t[:], in_=dsc_ext[:, :])
            idx_t = const.tile([P, iwsum], I16)
            nc.sync.dma_start(out=idx_t[:], in_=idx_ext[:, :])
            relw_t = const.tile([P, 2 * bsum], F32)
            nc.sync.dma_start(out=relw_t[:], in_=relw_ext[:, :])
            cb_t = None
            if include_c:
                cb_t = const.tile([P, D], F32)
                nc.sync.dma_start(out=cb_t[:], in_=cb_ext[:, :])

            loop_ctx = (tc.For_i(0, loop_n, 1) if loop_n > 1
                        else contextlib.nullcontext())
            with loop_ctx:
                # --- phase 1: m = LN(x) @ W2 (+c), all nodes, bf16 to HBM ---
                # G tiles (G*128 node rows) share one x-load / m-store DMA
                G = g_tiles
                for t0 in ([] if skip_p1 else range(0, nt, G)):
                    g_n = min(G, nt - t0)
                    xt4 = xp.tile([P, G, D], F32)
                    x_src = x_ext[t0 * P:(t0 + g_n) * P, :].rearrange(
                        "(j p) d -> p j d", p=P)
                    nc.sync.dma_start(out=xt4[:, :g_n, :], in_=x_src)
                    m4 = mp.tile([P, G, D], BF16)
                    for j in range(g_n):
                        xt = xt4[:, j, :]
                        st = sp.tile([P, 6], F32)
                        nc.vector.bn_stats(out=st[:], in_=xt)
                        mv = sp.tile([P, 2], F32)
                        nc.vector.bn_aggr(out=mv[:], in_=st[:])
                        sd = smp.tile([P, 1], F32)
                        nc.scalar.activation(out=sd[:], in_=mv[:, 1:2],
                                             func=AF.Sqrt, bias=eps_t[:, :],
                                             scale=1.0)
                        rstd = smp.tile([P, 1], F32)
                        nc.vector.reciprocal(out=rstd[:], in_=sd[:])
                        h = hp.tile([P, D], BF16)
                        nc.vector.tensor_scalar(out=h[:], in0=xt,
                                                scalar1=mv[:, 0:1],
                                                scalar2=rstd[:],
                                                op0=OP.subtract, op1=OP.mult)
                        ht_ps = ps_t.tile([P, D], BF16)
                        nc.tensor.transpose(out=ht_ps[:], in_=h[:],
                                            identity=ident[:])
                        ht = htp.tile([P, D], BF16)
                        if ht_copy_eng == "scalar":
                            nc.scalar.copy(out=ht[:], in_=ht_ps[:])
                        else:
                            nc.vector.tensor_copy(out=ht[:], in_=ht_ps[:])
                        m_ps = ps_m.tile([P, D], F32)
                        nc.tensor.matmul(out=m_ps[:], lhsT=ht[:], rhs=w2_t[:],
                                         start=True, stop=True)
                        if include_c:
                            nc.vector.tensor_add(out=m4[:, j, :], in0=m_ps[:],
                                                 in1=cb_t[:])
                        else:
                            nc.scalar.copy(out=m4[:, j, :], in_=m_ps[:])
                    m_dst = m_dram[t0 * P:(t0 + g_n) * P, :].rearrange(
                        "(j p) d -> p j d", p=P)
                    nc.sync.dma_start(out=m_dst, in_=m4[:, :g_n, :])

                # --- phase 2: per node-chunk gather + one-hot matmul ---
                for ci in ([] if skip_p2 else range(nch)):
                    lo, hi = b_lo[ci], b_hi[ci]
                    bci = lo + hi
                    iw0 = int(iwoff[ci])
                    b0 = int(boff[ci])
                    msg_lo = msgp.tile([P, bmax_lo, D], BF16, tag="msg_lo")
                    msg_hi = msgp.tile([P, bmax_hi, D], BF16, tag="msg_hi")
                    if lo and not skip_gather:
                        nc.gpsimd.dma_gather(
                            msg_lo[:, 0:lo, :], m_dram[0:half, :],
                            idx_t[:, iw0:iw0 + lo * 8],
                            lo * P, lo * P, D, single_packet=False,
                            queue_num=(2 * ci) % n_swdge)
                    if hi and not skip_gather:
                        nc.gpsimd.dma_gather(
                            msg_hi[:, 0:hi, :], m_dram[half:n_pad, :],
                            idx_t[:, iw0 + lo * 8:iw0 + (lo + hi) * 8],
                            hi * P, hi * P, D, single_packet=False,
                            queue_num=(2 * ci + 1) % n_swdge)

                    out_ps = ps_o.tile([P, D], F32)
                    deg_ps = ps_d.tile([P, 1], F32)
                    if skip_blocks:
                        nc.vector.memset(out_ps[:], 0.0)
                        nc.vector.memset(deg_ps[:], 1.0)
                    for b in ([] if skip_blocks else range(bci)):
                        oh = ohp.tile([P, P], BF16)
                        nc.vector.tensor_scalar(
                            out=oh[:], in0=iota_t[:],
                            scalar1=relw_t[:, b0 + b:b0 + b + 1],
                            scalar2=relw_t[:, bsum + b0 + b:bsum + b0 + b + 1],
                            op0=OP.is_equal, op1=OP.mult)
                        mrhs = (msg_lo[:, b, :] if b < lo
                                else msg_hi[:, b - lo, :])
                        nc.tensor.matmul(out=out_ps[:], lhsT=oh[:],
                                         rhs=mrhs,
                                         start=(b == 0), stop=(b == bci - 1))
                        nc.tensor.matmul(out=deg_ps[:], lhsT=oh[:],
                                         rhs=ones_t[:],
                                         start=(b == 0), stop=(b == bci - 1))

                    dmx = smp.tile([P, 1], F32, tag="dmx")
                    nc.vector.tensor_scalar(out=dmx[:], in0=deg_ps[:],
                                            scalar1=1.0, scalar2=None,
                                            op0=OP.max)
                    inv = smp.tile([P, 1], F32, tag="inv")
                    nc.vector.reciprocal(out=inv[:], in_=dmx[:])
                    sc = epp.tile([P, D], F32, tag="sc")
                    nc.vector.tensor_scalar(out=sc[:], in0=out_ps[:],
                                            scalar1=inv[:],
                                            scalar2=dsc_t[:, ci:ci + 1],
                                            op0=OP.mult, op1=OP.mult)
                    g = epp.tile([P, D], F32, tag="g")
                    nc.scalar.activation(out=g[:], in_=sc[:], func=AF.Gelu)
                    xr = epp.tile([P, D], F32, tag="xr")
                    nc.sync.dma_start(out=xr[:],
                                      in_=xres_ext[ci * P:(ci + 1) * P, :])
                    yt = epp.tile([P, D], F32, tag="yt")
                    nc.vector.tensor_add(out=yt[:], in0=g[:], in1=xr[:])
                    nc.sync.dma_start(out=y_ext[ci * P:(ci + 1) * P, :],
                                      in_=yt[:])

    return nc


def prepare_inputs(x, gamma, beta, W, b, edge_index, edge_weight, dst_scale,
                   n_cores):
    """Host-side sharding: sort edges by (dst-chunk, src-half), pad each
    (core, chunk, half) segment to whole 128-edge gather blocks."""
    N = x.shape[0]
    R = n_cores
    npc = (N + R - 1) // R
    nch = (npc + P - 1) // P
    npc_pad = nch * P
    n_pad = (((R - 1) * npc + npc_pad + P - 1) // P) * P
    if n_pad % 256:
        n_pad += 128  # need even halves for the int16 gather split
    half = n_pad // 2

    src = np.ascontiguousarray(edge_index[0]).astype(np.int64)
    dst = np.ascontiguousarray(edge_index[1]).astype(np.int64)
    w = edge_weight.astype(np.float32)
    E = src.shape[0]

    core_id = np.minimum(dst // npc, R - 1)
    local = dst - core_id * npc
    chunk_id = local // P
    rel = (local - chunk_id * P).astype(np.float32)
    hh = (src >= half).astype(np.int64)
    flat = (core_id * nch + chunk_id) * 2 + hh
    order = np.argsort(flat, kind="stable")
    flat_s = flat[order]
    src_s, rel_s, w_s, h_s = src[order], rel[order], w[order], hh[order]
    ci_s = (flat_s // 2) % nch
    r_s = flat_s // (2 * nch)

    cnt = np.bincount(flat_s, minlength=R * nch * 2).reshape(R, nch, 2)
    b_lo = tuple(int(v) for v in -(-cnt[:, :, 0].max(axis=0) // P))
    b_hi = tuple(int(v) for v in -(-cnt[:, :, 1].max(axis=0) // P))
    bc = np.array(b_lo) + np.array(b_hi)
    boff = np.concatenate([[0], np.cumsum(bc)])
    bsum = int(boff[-1])
    iw_per = (np.array(b_lo) + np.array(b_hi)) * 8
    iwoff = np.concatenate([[0], np.cumsum(iw_per)])
    iwsum = int(iwoff[-1])

    starts = np.searchsorted(flat_s, np.arange(R * nch * 2 + 1))
    pos = np.arange(E) - starts[flat_s]

    # rel/w in [P, bsum] block-column layout; idx wrapped int16 [16, iwsum]
    rel_arr = np.zeros((R, P, bsum), np.float32)
    w_arr = np.zeros((R, P, bsum), np.float32)
    idx_arr = np.zeros((R, 16, iwsum), np.int16)

    p_eg = pos % P
    j_eg = pos // P
    bglob = j_eg + h_s * np.array(b_lo)[ci_s]
    col_rw = boff[ci_s] + bglob
    rel_arr[r_s, p_eg, col_rw] = rel_s
    w_arr[r_s, p_eg, col_rw] = w_s
    colw = iwoff[ci_s] + h_s * (np.array(b_lo)[ci_s] * 8) + pos // 16
    idx_arr[r_s, pos % 16, colw] = (src_s - h_s * half).astype(np.int16)
    idx_full = np.ascontiguousarray(np.tile(idx_arr, (1, 8, 1)))
    relw = np.ascontiguousarray(np.concatenate([rel_arr, w_arr], axis=2))

    x_pad = np.zeros((n_pad, D), np.float32)
    x_pad[:N] = x.astype(np.float32)

    W2 = (W.T.astype(np.float32) * gamma.astype(np.float32)[:, None])
    W2 = W2.astype(ml_dtypes.bfloat16)
    c = beta.astype(np.float32) @ W.T.astype(np.float32) + b.astype(np.float32)
    include_c = bool(np.any(c != 0.0))
    cb = np.ascontiguousarray(np.broadcast_to(c, (P, D))).astype(np.float32)

    iota = np.broadcast_to(np.arange(P, dtype=np.float32), (P, P))
    iota = np.ascontiguousarray(iota).astype(ml_dtypes.bfloat16)
    ident = np.eye(P, dtype=np.float32).astype(ml_dtypes.bfloat16)

    in_maps = []
    for r in range(R):
        lo_n = r * npc
        hi_n = min(N, lo_n + npc)
        dsr = np.zeros(npc_pad, np.float32)
        dsr[:hi_n - lo_n] = dst_scale[lo_n:hi_n].astype(np.float32)
        dsct = np.ascontiguousarray(dsr.reshape(nch, P).T)
        xres = np.ascontiguousarray(x_pad[lo_n:lo_n + npc_pad])
        m = {
            "x": x_pad,
            "xres": xres,
            "w2": W2,
            "iota": iota,
            "ident": ident,
            "idx16": idx_full[r],
            "relw": relw[r],
            "dsct": dsct,
        }
        if include_c:
            m["cb"] = cb
        in_maps.append(m)
    geom = dict(n_pad=n_pad, nch=nch, include_c=include_c,
                b_lo=b_lo, b_hi=b_hi,
                npc=npc, npc_pad=npc_pad, N=N, R=R)
    return in_maps, geom


_PROGRAM_CACHE = {}


def kernel(x, gamma, beta, W, b, edge_index, num_nodes, edge_weight,
           dst_scale, n_cores=8, _collect=None):
    x = np.asarray(x)
    N = x.shape[0]
    in_maps, geom = prepare_inputs(
        np.asarray(x), np.asarray(gamma), np.asarray(beta), np.asarray(W),
        np.asarray(b), np.asarray(edge_index), np.asarray(edge_weight),
        np.asarray(dst_scale), n_cores)

    key = (geom["n_pad"], geom["nch"], geom["include_c"],
           geom["b_lo"], geom["b_hi"])
    nc = _PROGRAM_CACHE.get(key)
    if nc is None:
        nc = build_program(geom)
        nc.finalize()
        _PROGRAM_CACHE[key] = nc

    res = run_bass_kernel_spmd(nc, in_maps, list(range(n_cores)),
                               **(_collect.pop("kwargs") if _collect else {}))
    if _collect is not None:
        _collect["res"] = res

    y = np.empty((N, D), np.float32)
    npc = geom["npc"]
    for r in range(geom["R"]):
        lo = r * npc
        hi = min(N, lo + npc)
        y[lo:hi] = res.results[r]["y"][:hi - lo]
    return y
